# revision 2
# baseline (speedup 1.0000x reference)
"""Trainium2 Bass kernel for nn_CombinedHiddenEncoder (5-layer GCN stack on a
fixed random graph, N=50000 nodes, E=600000 edges + self loops).

Algebraic restructure: S = D^-1/2 (A+I) D^-1/2 is shared by all 5 GCNConvs and
commutes with right-multiplication, so with WMV = [Wm | Wv]:

    Y0     = T * (Xf @ (W1 W3a WMV) + Xc @ (W2 W3b WMV))      [N, 2*LD]
    P      = A01 T^2 A01 T^2 A01 Y0          (3 unweighted gather/scatter rounds)
    out    = T*(P + s-corrections)  ->  mean | logvar ; z = noise*exp(lv/2)+mean

All sparse-round tensors (tables, gathered rows, one-hot matrices) are bf16:
halves AllGather + gather HBM traffic and runs the one-hot scatter matmuls at
1 cycle/row instead of fp32's 4.  PSUM accumulation stays fp32.

Distribution: nodes sharded across 8 cores (6250 each, padded to 6272=49*128).
Each round: AllGather the bf16 node table [50176, 128], gather source rows for
the locally-owned dst edges (dst-sorted, 128-edge chunks), scatter-add via
  psum[dstloc, :] += onehot(dstloc).T @ gathered
TensorEngine matmuls.  dma_gather indices are int16, so edges are split by
source-row parity and gathered with elem_step=2 rows (idx = row >> 1).
Chunk counts are per-(tile, parity), sized to the max edge count across the 8
cores (SPMD: one program), not the global max -- less slack than uniform.
"""

import numpy as np

import concourse.bass as bass
import concourse.mybir as mybir
import concourse.tile as tile
from concourse import bacc
from concourse.bass_utils import run_bass_kernel_spmd

F32 = mybir.dt.float32
BF16 = mybir.dt.bfloat16
I16 = mybir.dt.int16

N, E = 50000, 600000
FD, CD, HD, LD = 256, 128, 128, 64
OD = 2 * LD                    # 128: [mean | logvar] feature width
CORES = 8
SHARD = N // CORES             # 6250
TILES = (SHARD + 127) // 128   # 49
R = TILES * 128                # 6272 padded rows per core
TR = CORES * R                 # 50176 table rows
GROUP = 7                      # dst-tiles per gather call
NGROUPS = TILES // GROUP       # 7

_prog_cache: dict = {}


# --------------------------------------------------------------------------
# Bass program builder
# --------------------------------------------------------------------------
def build_program(cpc, variant: str = "full"):
    """cpc[t][p] = number of 128-slot chunks for dst-tile t, parity p
    (identical across cores; lists padded host-side)."""
    do_gather = variant not in ("nogather",)
    do_mm = variant not in ("nomm",)
    do_cc = variant not in ("nocc",)
    n_rounds = int(variant[1:]) if variant.startswith("r") else 3
    nc = bacc.Bacc(None, target_bir_lowering=False)

    # per-call chunk counts and offsets; call order: (g, par)
    call_chunks = []            # chunks in call (g,p)
    for g in range(NGROUPS):
        for p in (0, 1):
            call_chunks.append(sum(cpc[g * GROUP + ti][p] for ti in range(GROUP)))
    ncalls = NGROUPS * 2
    total_chunks = sum(call_chunks)
    call_coff = np.concatenate([[0], np.cumsum(call_chunks)])  # chunk offsets
    # idx columns per call (idx wrapped into 16 partitions, replicated to 128)
    call_icols = [c * 128 // 16 for c in call_chunks]
    call_ioff = np.concatenate([[0], np.cumsum(call_icols)])
    total_icols = int(call_ioff[-1])

    # ---- I/O ----
    xfT = nc.dram_tensor("xfT", [FD, R], BF16, kind="ExternalInput")
    xcT = nc.dram_tensor("xcT", [CD, R], BF16, kind="ExternalInput")
    noise_in = nc.dram_tensor("noise_in", [R, LD], F32, kind="ExternalInput")
    aw = nc.dram_tensor("aw", [FD, OD], BF16, kind="ExternalInput")
    bw = nc.dram_tensor("bw", [CD, OD], BF16, kind="ExternalInput")
    cmv = nc.dram_tensor("cmv", [4, OD], BF16, kind="ExternalInput")
    dinv_p = nc.dram_tensor("dinv_p", [128, TILES], F32, kind="ExternalInput")
    dinv2_p = nc.dram_tensor("dinv2_p", [128, TILES], F32, kind="ExternalInput")
    srows = nc.dram_tensor("srows", [4, R], BF16, kind="ExternalInput")
    idx_all = nc.dram_tensor("idx_all", [128, total_icols], I16,
                             kind="ExternalInput")
    dstloc_all = nc.dram_tensor("dstloc_all", [128, total_chunks], F32,
                                kind="ExternalInput")

    z_out = nc.dram_tensor("z_out", [R, LD], F32, kind="ExternalOutput")
    mean_out = nc.dram_tensor("mean_out", [R, LD], F32, kind="ExternalOutput")
    logvar_out = nc.dram_tensor("logvar_out", [R, LD], F32, kind="ExternalOutput")

    # ---- internal DRAM ----
    bounce = [nc.dram_tensor(f"xb{r}", [R, OD], BF16) for r in range(3)]
    tabs = [nc.dram_tensor(f"tab{r}", [TR, OD], BF16, addr_space="Shared")
            for r in range(3)]
    rg = [list(range(CORES))]

    with tile.TileContext(nc) as tc:
        with tc.tile_pool(name="const", bufs=1) as cpool:
            colidx = cpool.tile([128, 128], BF16)
            nc.gpsimd.iota(colidx[:], pattern=[[1, 128]], base=0,
                           channel_multiplier=0,
                           allow_small_or_imprecise_dtypes=True)
            a0_s = cpool.tile([128, OD], BF16)
            a1_s = cpool.tile([128, OD], BF16)
            b_s = cpool.tile([128, OD], BF16)
            nc.sync.dma_start(out=a0_s[:], in_=aw[0:128, :])
            nc.sync.dma_start(out=a1_s[:], in_=aw[128:256, :])
            nc.sync.dma_start(out=b_s[:], in_=bw[:, :])
            cmv_s = cpool.tile([4, OD], BF16)
            nc.sync.dma_start(out=cmv_s[:4, :], in_=cmv[:, :])
            dinv_s = cpool.tile([128, TILES], F32)
            dinv2_s = cpool.tile([128, TILES], F32)
            nc.sync.dma_start(out=dinv_s[:], in_=dinv_p[:, :])
            nc.sync.dma_start(out=dinv2_s[:], in_=dinv2_p[:, :])
            idx_s = cpool.tile([128, total_icols], I16)
            nc.sync.dma_start(out=idx_s[:], in_=idx_all[:, :])
            dloc_s = cpool.tile([128, total_chunks], F32)
            nc.sync.dma_start(out=dloc_s[:], in_=dstloc_all[:, :])

            with tc.tile_pool(name="psum", bufs=6, space="PSUM") as mmpool:

                # ---------------- stage 0: Y0 = T (Xf A + Xc B) --------------
                with tc.tile_pool(name="s0", bufs=1) as s0pool:
                    xf0_s = s0pool.tile([128, R], BF16)
                    xf1_s = s0pool.tile([128, R], BF16)
                    xc_s = s0pool.tile([128, R], BF16)
                    x0strip = s0pool.tile([128, R], BF16)
                    nc.sync.dma_start(out=xf0_s[:], in_=xfT[0:128, :])
                    nc.sync.dma_start(out=xf1_s[:], in_=xfT[128:256, :])
                    nc.sync.dma_start(out=xc_s[:], in_=xcT[:, :])
                    for t in range(TILES):
                        cs = slice(t * 128, (t + 1) * 128)
                        ps = mmpool.tile([128, OD], F32, name=f"s0ps{t}",
                                         tag="mm")
                        nc.tensor.matmul(ps[:], xf0_s[:, cs], a0_s[:],
                                         start=True, stop=False)
                        nc.tensor.matmul(ps[:], xf1_s[:, cs], a1_s[:],
                                         start=False, stop=False)
                        nc.tensor.matmul(ps[:], xc_s[:, cs], b_s[:],
                                         start=False, stop=True)
                        nc.vector.tensor_scalar(
                            out=x0strip[:, cs], in0=ps[:],
                            scalar1=dinv_s[:, t:t + 1], scalar2=None,
                            op0=mybir.AluOpType.mult)
                    nc.sync.dma_start(
                        out=bounce[0].ap().rearrange("(t p) h -> p t h", p=128),
                        in_=x0strip[:].rearrange("p (t h) -> p t h", h=OD))

                if do_cc:
                    nc.gpsimd.collective_compute(
                        "AllGather", mybir.AluOpType.bypass, replica_groups=rg,
                        ins=[bounce[0].ap()], outs=[tabs[0].ap()])

                # ---------------- 3 sparse rounds ---------------------------
                with tc.tile_pool(name="rnd", bufs=1) as rpool, \
                     tc.tile_pool(name="gpool", bufs=4) as gpool, \
                     tc.tile_pool(name="qpool", bufs=8) as qpool, \
                     tc.tile_pool(name="hd", bufs=3) as hdpool:
                    xstrip = rpool.tile([128, R], BF16)
                    for rnd in range(n_rounds):
                        tab = tabs[rnd]
                        tab2 = tab.ap().rearrange("(r two) h -> r two h", two=2)
                        last = rnd == n_rounds - 1
                        for g in range(NGROUPS):
                            gts = []
                            for par in (0, 1):
                                call = g * 2 + par
                                nchunks = call_chunks[call]
                                gt = gpool.tile([128, nchunks, 128], BF16,
                                                name=f"gt{rnd}_{g}_{par}",
                                                tag="gath")
                                icols = slice(int(call_ioff[call]),
                                              int(call_ioff[call + 1]))
                                if not do_gather:
                                    nc.vector.tensor_scalar(
                                        out=gt[:, 0, :], in0=colidx[:],
                                        scalar1=1.0, scalar2=None,
                                        op0=mybir.AluOpType.mult)
                                if do_gather:
                                    nc.gpsimd.dma_gather(
                                        out_ap=gt[:],
                                        in_ap=tab2[:, par, :],
                                        idxs_ap=idx_s[:, icols],
                                        num_idxs=nchunks * 128,
                                        num_idxs_reg=nchunks * 128,
                                        elem_size=OD,
                                        elem_step=2 * OD,
                                        single_packet=False)
                                gts.append(gt)
                            # chunk offset of tile ti within call, per parity
                            off = [0, 0]
                            for ti in range(GROUP):
                                t = g * GROUP + ti
                                ps = mmpool.tile([128, OD], F32,
                                                 name=f"ps{rnd}_{t}", tag="mm")
                                nmm = (cpc[t][0] + cpc[t][1]) if do_mm else 1
                                if last:
                                    nmm += 1    # correction matmul
                                k = 0
                                for par in ((0, 1) if do_mm else (0,)):
                                    nch = cpc[t][par] if do_mm else 1
                                    for c in range(nch):
                                        col = int(call_coff[g * 2 + par]) \
                                            + off[par] + c
                                        q = qpool.tile([128, 128], BF16,
                                                       name=f"q{rnd}_{t}_{k}",
                                                       tag="q")
                                        nc.vector.tensor_scalar(
                                            out=q[:], in0=colidx[:],
                                            scalar1=dloc_s[:, col:col + 1],
                                            scalar2=None,
                                            op0=mybir.AluOpType.is_equal)
                                        nc.tensor.matmul(
                                            ps[:], q[:],
                                            gts[par][:, off[par] + c, :],
                                            start=(k == 0),
                                            stop=(k == nmm - 1))
                                        k += 1
                                if do_mm:
                                    off[0] += cpc[t][0]
                                    off[1] += cpc[t][1]
                                cs = slice(t * 128, (t + 1) * 128)
                                if not last:
                                    nc.vector.tensor_scalar(
                                        out=xstrip[:, cs], in0=ps[:],
                                        scalar1=dinv2_s[:, t:t + 1],
                                        scalar2=None,
                                        op0=mybir.AluOpType.mult)
                                else:
                                    # correction: ps += sr[:3,cs]^T @ cmv[:3]
                                    sr = hdpool.tile([4, 128], BF16,
                                                     name=f"sr{t}", tag="sr")
                                    nc.sync.dma_start(out=sr[:4, :],
                                                      in_=srows[:, cs])
                                    nc.tensor.matmul(ps[:], sr[:3, :],
                                                     cmv_s[:3, :],
                                                     start=(k == 0), stop=True)
                                    nz = hdpool.tile([128, LD], F32,
                                                     name=f"nz{t}", tag="nz")
                                    nc.sync.dma_start(out=nz[:],
                                                      in_=noise_in[cs, :])
                                    mnlv = hdpool.tile([128, OD], F32,
                                                       name=f"mnlv{t}",
                                                       tag="mnlv")
                                    nc.vector.tensor_scalar(
                                        out=mnlv[:], in0=ps[:],
                                        scalar1=dinv_s[:, t:t + 1],
                                        scalar2=None,
                                        op0=mybir.AluOpType.mult)
                                    ex = hdpool.tile([128, LD], F32,
                                                     name=f"ex{t}", tag="ex")
                                    nc.scalar.activation(
                                        out=ex[:], in_=mnlv[:, LD:OD],
                                        func=mybir.ActivationFunctionType.Exp,
                                        scale=0.5)
                                    zt = hdpool.tile([128, LD], F32,
                                                     name=f"zt{t}", tag="zt")
                                    nc.vector.tensor_tensor(
                                        out=zt[:], in0=nz[:], in1=ex[:],
                                        op=mybir.AluOpType.mult)
                                    nc.vector.tensor_tensor(
                                        out=zt[:], in0=zt[:],
                                        in1=mnlv[:, 0:LD],
                                        op=mybir.AluOpType.add)
                                    nc.sync.dma_start(out=z_out[cs, :],
                                                      in_=zt[:])
                                    nc.sync.dma_start(out=mean_out[cs, :],
                                                      in_=mnlv[:, 0:LD])
                                    nc.sync.dma_start(out=logvar_out[cs, :],
                                                      in_=mnlv[:, LD:OD])
                        if not last:
                            nc.sync.dma_start(
                                out=bounce[rnd + 1].ap().rearrange(
                                    "(t p) h -> p t h", p=128),
                                in_=xstrip[:].rearrange(
                                    "p (t h) -> p t h", h=OD))
                            if do_cc:
                                nc.gpsimd.collective_compute(
                                    "AllGather", mybir.AluOpType.bypass,
                                    replica_groups=rg,
                                    ins=[bounce[rnd + 1].ap()],
                                    outs=[tabs[rnd + 1].ap()])
    nc.finalize()
    return nc


# --------------------------------------------------------------------------
# Host-side preprocessing
# --------------------------------------------------------------------------
def preprocess(feature, condition, edge_index, noise,
               W1, b1, W2, b2, W3, b3, Wm, bm, Wv, bv):
    feature = np.asarray(feature, np.float32)
    condition = np.asarray(condition, np.float32)
    noise = np.asarray(noise, np.float32)
    ei = np.asarray(edge_index).astype(np.int64)
    W1 = np.asarray(W1, np.float32); b1 = np.asarray(b1, np.float32)
    W2 = np.asarray(W2, np.float32); b2 = np.asarray(b2, np.float32)
    W3 = np.asarray(W3, np.float32); b3 = np.asarray(b3, np.float32)
    Wm = np.asarray(Wm, np.float32); bm = np.asarray(bm, np.float32)
    Wv = np.asarray(Wv, np.float32); bv = np.asarray(bv, np.float32)

    loop = np.arange(N, dtype=np.int64)
    src = np.concatenate([ei[0], loop])
    dst = np.concatenate([ei[1], loop])
    deg = np.bincount(dst, minlength=N).astype(np.float64)
    dinv = 1.0 / np.sqrt(deg)
    w = dinv[src] * dinv[dst]
    s1 = np.bincount(dst, weights=w, minlength=N)
    s2 = np.bincount(dst, weights=w * s1[src], minlength=N)
    dinv32 = dinv.astype(np.float32)

    WMV = np.concatenate([Wm, Wv], axis=1)              # [HD, OD]
    W3a, W3b = W3[:HD], W3[HD:]
    A_w = (W1 @ W3a @ WMV).astype(np.float32)           # [FD, OD]
    B_w = (W2 @ W3b @ WMV).astype(np.float32)           # [CD, OD]
    c1 = b1 @ W3a + b2 @ W3b
    Cmv = np.zeros((4, OD), np.float32)
    Cmv[0] = c1 @ WMV
    Cmv[1] = b3 @ WMV
    Cmv[2] = np.concatenate([bm, bv])

    node = np.arange(N, dtype=np.int64)
    pos_of_node = (node // SHARD) * R + (node % SHARD)
    pos_src = pos_of_node[src]

    core = dst // SHARD
    d_loc = dst - core * SHARD
    tl = d_loc // 128
    dstloc = d_loc % 128
    parity = pos_src & 1
    idx16 = (pos_src >> 1).astype(np.int64)

    # counts per (core, tile, parity) -> shared chunk counts cpc[t][p]
    gid = (core * TILES + tl) * 2 + parity
    ngroups_tot = CORES * TILES * 2
    counts = np.bincount(gid, minlength=ngroups_tot).reshape(CORES, TILES, 2)
    cpc = np.ceil(counts.max(axis=0) / 128).astype(np.int64)   # [TILES, 2]
    cpc_t = tuple(tuple(int(x) for x in row) for row in cpc)

    # slot layout: chunks ordered by call (g, par), tile within group, chunk
    # global chunk id for (t, p): coff[t, p]
    coff = np.zeros((TILES, 2), np.int64)
    acc = 0
    for g in range(NGROUPS):
        for p in (0, 1):
            for ti in range(GROUP):
                t = g * GROUP + ti
                coff[t, p] = acc
                acc += cpc[t, p]
    total_chunks = acc
    slot_base = coff * 128                     # slot offset of (t, p)

    order = np.lexsort((parity, tl, core))     # sort by (core, tile, parity)
    # within-group position
    gid_sorted = gid[order]
    counts_flat = counts.reshape(-1)
    starts = np.concatenate([[0], np.cumsum(counts_flat)[:-1]])
    within = np.arange(len(gid_sorted)) - np.repeat(starts, counts_flat)

    # per-core slot arrays
    idx_slots = np.zeros((CORES, total_chunks * 128), np.int16)
    dl_slots = np.full((CORES, total_chunks * 128), -1.0, np.float32)
    core_sorted = core[order]
    t_sorted = tl[order]
    p_sorted = parity[order]
    slot = slot_base[t_sorted, p_sorted] + within
    idx_slots[core_sorted, slot] = idx16[order].astype(np.int16)
    dl_slots[core_sorted, slot] = dstloc[order].astype(np.float32)

    # call boundaries in chunk space
    call_chunks = []
    for g in range(NGROUPS):
        for p in (0, 1):
            call_chunks.append(int(cpc[g * GROUP: (g + 1) * GROUP, p].sum()))
    call_coff = np.concatenate([[0], np.cumsum(call_chunks)]).astype(np.int64)

    in_maps = []
    for k in range(CORES):
        rows = slice(k * SHARD, (k + 1) * SHARD)
        xfT = np.zeros((FD, R), np.float32)
        xfT[:, :SHARD] = feature[rows].T
        xcT = np.zeros((CD, R), np.float32)
        xcT[:, :SHARD] = condition[rows].T
        nz = np.zeros((R, LD), np.float32)
        nz[:SHARD] = noise[rows]
        dv = np.zeros((TILES, 128), np.float32)
        dv.reshape(-1)[:SHARD] = dinv32[rows]
        sr = np.zeros((4, R), np.float32)
        di = dinv[rows.start:rows.stop]
        sr[0, :SHARD] = (s2[rows] / di).astype(np.float32)
        sr[1, :SHARD] = (s1[rows] / di).astype(np.float32)
        sr[2, :SHARD] = (1.0 / di).astype(np.float32)

        # idx per call: wrap each call's idx list into 16 partitions, rep x8
        parts = []
        for call in range(NGROUPS * 2):
            s0 = int(call_coff[call]) * 128
            s1_ = int(call_coff[call + 1]) * 128
            ic = idx_slots[k, s0:s1_].reshape(-1, 16)      # [nidx/16, 16]
            parts.append(ic.T)                             # [16, nidx/16]
        idx_arr = np.tile(np.concatenate(parts, axis=1), (8, 1))

        dl_arr = np.ascontiguousarray(
            dl_slots[k].reshape(total_chunks, 128).T)      # [128, chunks]

        def bf(x):
            try:
                import ml_dtypes
                return np.asarray(x, np.float32).astype(ml_dtypes.bfloat16)
            except ImportError:
                import jax.numpy as jnp
                return np.asarray(jnp.asarray(x, jnp.bfloat16))

        in_maps.append({
            "xfT": bf(xfT), "xcT": bf(xcT), "noise_in": nz,
            "aw": bf(A_w), "bw": bf(B_w), "cmv": bf(Cmv),
            "dinv_p": np.ascontiguousarray(dv.T),
            "dinv2_p": np.ascontiguousarray((dv ** 2).T),
            "srows": bf(sr),
            "idx_all": np.ascontiguousarray(idx_arr),
            "dstloc_all": dl_arr,
        })
    return cpc_t, in_maps


def kernel(feature, condition, edge_index, noise,
           W1, b1, W2, b2, W3, b3, Wm, bm, Wv, bv, _trace=False):
    cpc_t, in_maps = preprocess(feature, condition, edge_index, noise,
                                W1, b1, W2, b2, W3, b3, Wm, bm, Wv, bv)
    if cpc_t not in _prog_cache:
        _prog_cache[cpc_t] = build_program(cpc_t)
    nc = _prog_cache[cpc_t]
    res = run_bass_kernel_spmd(nc, in_maps, list(range(CORES)), trace=_trace)
    z = np.concatenate([res.results[k]["z_out"][:SHARD] for k in range(CORES)])
    mean = np.concatenate(
        [res.results[k]["mean_out"][:SHARD] for k in range(CORES)])
    logvar = np.concatenate(
        [res.results[k]["logvar_out"][:SHARD] for k in range(CORES)])
    if _trace:
        kernel._last_exec_time_ns = res.exec_time_ns
        kernel._last_results = res
    return (z, mean, logvar)
